# revision 7
# baseline (speedup 1.0000x reference)
"""Trainium2 Bass kernel for retrieval_knn (nn_Direct_25701084299719).

For each of N=4096 query points vs M=16384 voxels:
  - top-8 nearest voxels (L2), mean of their normals
  - cosine(mean_normal, voxel_normal) > 0.75 mask
  - score_num = sum(mask); score_sum = sum(score * mask / exp(distance))
  - out = (score_sum / max(score_num, 1)) where score_num != 0 else 0, plus valid

Sharding: data-parallel over queries across 8 NeuronCores (512 queries/core);
voxel tensors replicated. No collectives.

Device algorithm (per core; selection in fp32, streaming math in bf16):
  y[q,m]    = 2<x_q, v_m> - |v_m|^2          (K=4 f32r matmul; desc == nearest)
  top8/idx  = nc.vector.max / max_index      (native per-partition top-8)
  xn        = sum of 8 gathered normals      (indirect-DMA gather, strided adds)
  numth[q,m]= <xn_q, n_m> - 0.75|xn_q||n_m|  (second K=4 f32r matmul)
  mask      = numth > 0  (ACT Sign, count fused via accum_out)
  contrib   = sign * exp(ln s - d), fused multiply+reduce on DVE

Engine split per 1024-col PSUM chunk:
  PE: 2x 512-col f32r matmul   Pool: PSUM->y copy (f32, for selection)
  ACT: Sqrt direct from PSUM   DVE: z = ln s - d (bf16 2x)
then one full-row Exp (accum -> sum es), and in phase C per chunk:
  ACT: Sign from PSUM (accum -> count)   DVE: scalar_tensor_tensor
  prod+partial-reduce (bf16, accum -> sum sg*es)
"""

import sys

for p in ("/opt/trn_rl_repo", "/root/.axon_site/_ro/trn_rl_repo"):
    if p not in sys.path:
        sys.path.insert(0, p)

import numpy as np
from contextlib import ExitStack

import concourse.bass as bass
import concourse.mybir as mybir
from concourse import tile, masks, bacc
from concourse.bass import IndirectOffsetOnAxis
from concourse.bass_utils import run_bass_kernel_spmd

F32 = mybir.dt.float32
BF16 = mybir.dt.bfloat16
F32R = mybir.dt.float32r
U32 = mybir.dt.uint32
AF = mybir.ActivationFunctionType
OP = mybir.AluOpType
AX = mybir.AxisListType

N, M, K = 4096, 16384, 8
NCORES = 8
NQ = N // NCORES          # 512 queries per core
NT = NQ // 128            # 4 query tiles per core
CH = 1024                 # m-chunk (2 PSUM banks)
NCH = M // CH             # 16 chunks
LCH = 4096                # A-table DMA load chunk
NLCH = M // LCH           # 4 loads per table per tile

_nc_cache = {}


def build_nc():
    if "nc" in _nc_cache:
        return _nc_cache["nc"]
    nc = bacc.Bacc("TRN2", target_bir_lowering=False, debug=False)
    x_d = nc.declare_dram_parameter("x", [NQ, 3], F32, isOutput=False)
    vp_d = nc.declare_dram_parameter("voxel_point", [M, 3], F32, isOutput=False)
    vn_d = nc.declare_dram_parameter("voxel_normal", [M, 3], F32, isOutput=False)
    sc_d = nc.declare_dram_parameter("score", [M], F32, isOutput=False)
    out_d = nc.declare_dram_parameter("out", [128, 2 * NT], F32, isOutput=True)
    # Internal DRAM staging. All m-indexed tensors share one fixed voxel
    # permutation m' = j*128+p <-> voxel p*128+j (from the PE-transpose build);
    # reductions over m are permutation-invariant so results are unchanged.
    Adist = nc.dram_tensor("A_dist", [4, M], F32)
    Acos = nc.dram_tensor("A_cos", [4, M], F32)
    vn4_dram = nc.dram_tensor("vn4", [M, 4], F32)     # padded normals for gather
    sc16_dram = nc.dram_tensor("sc16", [M], BF16)     # permuted bf16 ln(score)

    with tile.TileContext(nc) as tc, ExitStack() as ctx:
        cpool = ctx.enter_context(tc.tile_pool(name="const", bufs=1))
        big = ctx.enter_context(tc.tile_pool(name="big", bufs=1))
        prep = ctx.enter_context(tc.tile_pool(name="prep", bufs=2))
        chk = ctx.enter_context(tc.tile_pool(name="chk", bufs=2))
        sgp = ctx.enter_context(tc.tile_pool(name="sgp", bufs=3))
        ppb = ctx.enter_context(tc.tile_pool(name="ppb", bufs=3, space="PSUM"))
        ppt = ctx.enter_context(tc.tile_pool(name="ppt", bufs=2, space="PSUM"))
        sm = ctx.enter_context(tc.tile_pool(name="sm", bufs=2))

        ident = cpool.tile([128, 128], F32)
        masks.make_identity(nc, ident[:])

        # ---------------- prep: voxel-side tensors ----------------
        vn_sb = prep.tile([128, 384], F32)
        nc.sync.dma_start(vn_sb[:], vn_d[:].rearrange("(p j) d -> p (j d)", p=128))
        vp_sb = prep.tile([128, 384], F32)
        nc.sync.dma_start(vp_sb[:], vp_d[:].rearrange("(p j) d -> p (j d)", p=128))

        def build_A(src_sb, A_dram, row3_from_sq, keep4_sb=None):
            """A rows 0-2: coordinate d in m' order; row 3 from sum of squares.
            keep4_sb: optionally collect rows 0-2 into a [128, 128*4] row-padded
            layout (gather table: partition j holds voxels j*128..j*128+127)."""
            v3 = src_sb[:].rearrange("p (j d) -> p j d", d=3)
            for d in range(3):
                ps = ppt.tile([128, 128], F32, tag="pt")
                nc.tensor.transpose(ps[:], v3[:, :, d], ident[:])
                tmp = prep.tile([128, 128], F32, tag="atmp")
                nc.scalar.activation(tmp[:], ps[:], AF.Copy)
                nc.sync.dma_start(
                    A_dram[d : d + 1, :].rearrange("o (j p) -> (o j) p", j=128), tmp[:]
                )
                if keep4_sb is not None:
                    k4 = keep4_sb[:].rearrange("p (c e) -> p c e", e=4)
                    nc.vector.tensor_copy(k4[:, :, d], tmp[:])
            sq = prep.tile([128, 384], F32, tag="asq")
            nc.scalar.activation(sq[:], src_sb[:], AF.Square)
            s3 = sq[:].rearrange("p (j d) -> p j d", d=3)
            ss = prep.tile([128, 128], F32, tag="ass")
            nc.vector.tensor_tensor(ss[:], s3[:, :, 0], s3[:, :, 1], OP.add)
            nc.vector.tensor_tensor(ss[:], ss[:], s3[:, :, 2], OP.add)
            r3 = prep.tile([128, 128], F32, tag="ar3")
            row3_from_sq(r3, ss)
            ps = ppt.tile([128, 128], F32, tag="pt")
            nc.tensor.transpose(ps[:], r3[:], ident[:])
            tmp = prep.tile([128, 128], F32, tag="atmp")
            nc.scalar.activation(tmp[:], ps[:], AF.Copy)
            nc.sync.dma_start(
                A_dram[3:4, :].rearrange("o (j p) -> (o j) p", j=128), tmp[:]
            )

        # A_dist row3 = -|v|^2 ;  A_cos row3 = +0.75*|n| = sqrt(0.5625*|n|^2)
        build_A(vp_sb, Adist,
                lambda r3, ss: nc.vector.tensor_scalar_mul(r3[:], ss[:], -1.0))
        vn4_sb = prep.tile([128, 512], F32)
        nc.vector.memset(vn4_sb[:], 0.0)
        build_A(vn_sb, Acos,
                lambda r3, ss: nc.scalar.activation(r3[:], ss[:], AF.Sqrt, scale=0.5625),
                keep4_sb=vn4_sb)
        # vn4[j*128+p, 0:3] = vn in m' order, rows padded to 16B for the gather
        nc.sync.dma_start(
            vn4_dram[:].rearrange("(j p) e -> j (p e)", j=128), vn4_sb[:]
        )

        # bf16 permuted ln(score) row in DRAM, then partition-broadcast.
        # Folding score into the exponent (exp(ln s - d)) removes one full
        # elementwise multiply from the stream.
        sc_pm = prep.tile([128, 128], F32)
        nc.sync.dma_start(sc_pm[:], sc_d[:].rearrange("(p j) -> p j", p=128))
        ln_pm = prep.tile([128, 128], F32)
        nc.scalar.activation(ln_pm[:], sc_pm[:], AF.Ln)
        psT = ppt.tile([128, 128], F32, tag="pt")
        nc.tensor.transpose(psT[:], ln_pm[:], ident[:])
        scT16 = prep.tile([128, 128], BF16)
        nc.scalar.activation(scT16[:], psT[:], AF.Copy)
        nc.sync.dma_start(sc16_dram[:].rearrange("(j p) -> j p", j=128), scT16[:])
        lns_bc = big.tile([128, M], BF16, tag="sbc")
        nc.sync.dma_start(
            lns_bc[:], sc16_dram[:].rearrange("(o m) -> o m", o=1).partition_broadcast(128)
        )

        # ---------------- prep: query-side tensors ----------------
        xxs, lts = [], []
        for t in range(NT):
            xt = cpool.tile([128, 3], F32, tag=f"xt{t}")
            nc.sync.dma_start(xt[:], x_d[t * 128 : (t + 1) * 128, :])
            sqx = sm.tile([128, 3], F32, tag="sqx")
            nc.scalar.activation(sqx[:], xt[:], AF.Square)
            xx = cpool.tile([128, 1], F32, tag=f"xx{t}")
            nc.vector.tensor_reduce(xx[:], sqx[:], AX.X, OP.add)
            # lhsT rows [2x0;2x1;2x2;1]: 0.5 in col 3 pre-transpose, Copy(scale=2)
            xt4 = sm.tile([128, 4], F32, tag="xt4")
            nc.vector.tensor_copy(xt4[:, 0:3], xt[:])
            nc.vector.memset(xt4[:, 3:4], 0.5)
            ps = ppt.tile([128, 128], F32, tag="pt")
            nc.tensor.transpose(ps[0:4, 0:128], xt4[:], ident[:])
            lt = cpool.tile([4, 128], F32, tag=f"lt{t}")
            nc.scalar.activation(lt[:], ps[0:4, 0:128], AF.Copy, scale=2.0)
            xxs.append(xx)
            lts.append(lt)

        out_sb = cpool.tile([128, 2 * NT], F32)

        # ---------------- main loop over query tiles ----------------
        for t in range(NT):
            # Phase A: y = 2<x,v> - |v|^2 (f32r matmul), streamed per chunk:
            #   Pool copies PSUM->y (selection input), ACT computes
            #   d = sqrt(xx - y) straight from PSUM, DVE z = ln s - d.
            y = big.tile([128, M], F32, tag="y")
            dfull = big.tile([128, M], BF16, tag="dfull")
            for lc in range(NLCH):
                # f32 (not f32r) on the dist matmul: f32r is tf32-like and its
                # noise exceeds min-dist^2, making sqrt(xx - y_max) go NaN.
                ra = chk.tile([4, LCH], F32, tag="ra")
                nc.sync.dma_start(
                    ra[:], Adist[:, lc * LCH : (lc + 1) * LCH]
                )
                for cc in range(LCH // CH):
                    c = lc * (LCH // CH) + cc
                    cs = slice(c * CH, (c + 1) * CH)
                    ps = ppb.tile([128, CH], F32, tag="pm")
                    for h in range(2):
                        nc.tensor.matmul(
                            ps[:, h * 512 : (h + 1) * 512],
                            lhsT=lts[t][:],
                            rhs=ra[:, cc * CH + h * 512 : cc * CH + (h + 1) * 512],
                            start=True, stop=True,
                        )
                    # Pool/GpSimd and DMA can't read PSUM; alternate the
                    # selection copy between ACT and DVE to balance load.
                    # Sqrt+bias must read SBUF: the PSUM-input activation
                    # drops the bias on HW (sim diverges), giving sqrt(-y).
                    if c % 2 == 0:
                        nc.scalar.activation(y[:, cs], ps[:], AF.Copy)
                    else:
                        nc.vector.tensor_copy(y[:, cs], ps[:])
                    nc.scalar.activation(
                        dfull[:, cs], y[:, cs], AF.Sqrt, bias=xxs[t][:], scale=-1.0
                    )
                    nc.vector.tensor_tensor(
                        dfull[:, cs], lns_bc[:, cs], dfull[:, cs], OP.subtract
                    )

            # One full-row exp: es = s * exp(-d); accE = sum(es) for sign trick
            accE = sm.tile([128, 1], F32, tag="accE")
            nc.scalar.activation(dfull[:], dfull[:], AF.Exp, accum_out=accE[:])

            # Selection: native top-8 + indices, gather padded normals
            top8 = sm.tile([128, 8], F32, tag="top8")
            nc.vector.max(top8[:], y[:])
            idx8 = sm.tile([128, 8], U32, tag="idx8")
            nc.vector.max_index(idx8[:], top8[:], y[:])
            # HW DGE consumes one offset per partition per instruction, so
            # issue one gather per neighbor rank.
            g = sm.tile([128, 32], F32, tag="gat")
            g3 = g[:].rearrange("p (i e) -> p i e", e=4)
            for i in range(8):
                nc.gpsimd.indirect_dma_start(
                    g3[:, i, :], None, vn4_dram[:],
                    IndirectOffsetOnAxis(ap=idx8[:, i : i + 1], axis=0),
                )
            # xn = sum of the 8 gathered normal rows (tree of strided adds)
            h16 = sm.tile([128, 16], F32, tag="h16")
            nc.vector.tensor_tensor(h16[:], g[:, 0:16], g[:, 16:32], OP.add)
            h8 = sm.tile([128, 8], F32, tag="h8")
            nc.vector.tensor_tensor(h8[:], h16[:, 0:8], h16[:, 8:16], OP.add)
            xn4 = sm.tile([128, 4], F32, tag="xn4")
            nc.vector.tensor_tensor(xn4[:], h8[:, 0:4], h8[:, 4:8], OP.add)

            # lhsT for cos matmul: [xn0;xn1;xn2;-|xn|] (xn = 8*mean, scale-free)
            sqn = sm.tile([128, 3], F32, tag="sqn")
            nc.vector.tensor_tensor(sqn[:], xn4[:, 0:3], xn4[:, 0:3], OP.mult)
            nrm2 = sm.tile([128, 1], F32, tag="nrm2")
            nc.vector.tensor_reduce(nrm2[:], sqn[:], AX.X, OP.add)
            nc.scalar.activation(xn4[:, 3:4], nrm2[:], AF.Sqrt)
            nc.vector.tensor_scalar_mul(xn4[:, 3:4], xn4[:, 3:4], -1.0)
            psl = ppt.tile([128, 128], F32, tag="pt")
            nc.tensor.transpose(psl[0:4, 0:128], xn4[:], ident[:])
            ltc = sm.tile([4, 128], F32R, tag="ltc")
            nc.vector.tensor_copy(ltc[:], psl[0:4, 0:128])

            # Phase C per chunk: cos matmul -> ACT Sign (accum: count) ->
            # DVE fused prod+partial-reduce (accum: sum sg*es).
            # count = (sum(sg) + M)/2 ; ssum = (sum(sg*es) + sum(es))/2
            accSg = sm.tile([128, NCH], F32, tag="accSg")
            accS = sm.tile([128, NCH], F32, tag="accS")
            for lc in range(NLCH):
                rc = chk.tile([4, LCH], F32R, tag="ra")
                nc.sync.dma_start(
                    rc[:], Acos[:, lc * LCH : (lc + 1) * LCH].bitcast(F32R)
                )
                for cc in range(LCH // CH):
                    c = lc * (LCH // CH) + cc
                    cs = slice(c * CH, (c + 1) * CH)
                    psn = ppb.tile([128, CH], F32, tag="pm")
                    for h in range(2):
                        nc.tensor.matmul(
                            psn[:, h * 512 : (h + 1) * 512],
                            lhsT=ltc[:],
                            rhs=rc[:, cc * CH + h * 512 : cc * CH + (h + 1) * 512],
                            start=True, stop=True,
                        )
                    sg = sgp.tile([128, CH], BF16, tag="sg")
                    nc.scalar.activation(
                        sg[:], psn[:], AF.Sign, accum_out=accSg[:, c : c + 1]
                    )
                    prod = sgp.tile([128, CH], BF16, tag="prod")
                    nc.vector.scalar_tensor_tensor(
                        prod[:], sg[:], 1.0, dfull[:, cs], OP.mult, OP.mult,
                        accum_out=accS[:, c : c + 1],
                    )

            cnt = sm.tile([128, 1], F32, tag="cnt")
            nc.vector.tensor_reduce(cnt[:], accSg[:], AX.X, OP.add)
            nc.vector.tensor_scalar(cnt[:], cnt[:], float(M), 0.5, OP.add, OP.mult)
            ssum = sm.tile([128, 1], F32, tag="ssum")
            nc.vector.tensor_reduce(ssum[:], accS[:], AX.X, OP.add)
            nc.vector.tensor_tensor(ssum[:], ssum[:], accE[:], OP.add)
            nc.vector.tensor_scalar_mul(ssum[:], ssum[:], 0.5)
            den = sm.tile([128, 1], F32, tag="den")
            nc.vector.tensor_scalar_max(den[:], cnt[:], 1.0)
            rden = sm.tile([128, 1], F32, tag="rden")
            nc.vector.reciprocal(rden[:], den[:])
            vld = sm.tile([128, 1], F32, tag="vld")
            nc.vector.tensor_scalar(vld[:], cnt[:], 0.5, None, OP.is_gt)
            fld = sm.tile([128, 1], F32, tag="fld")
            nc.vector.tensor_tensor(fld[:], ssum[:], rden[:], OP.mult)
            nc.vector.tensor_tensor(fld[:], fld[:], vld[:], OP.mult)
            nc.vector.tensor_copy(out_sb[:, t : t + 1], fld[:])
            nc.vector.tensor_copy(out_sb[:, NT + t : NT + t + 1], cnt[:])

        nc.sync.dma_start(out_d[:, :], out_sb[:])

    nc.compile()
    _nc_cache["nc"] = nc
    return nc


def make_in_maps(x_world, voxel_point, voxel_normal, score):
    x = np.ascontiguousarray(np.asarray(x_world, np.float32).reshape(N, 3))
    vp = np.ascontiguousarray(np.asarray(voxel_point, np.float32).reshape(M, 3))
    vn = np.ascontiguousarray(np.asarray(voxel_normal, np.float32).reshape(M, 3))
    sc = np.ascontiguousarray(np.asarray(score, np.float32).reshape(M))
    return [
        {
            "x": np.ascontiguousarray(x[c * NQ : (c + 1) * NQ]),
            "voxel_point": vp,
            "voxel_normal": vn,
            "score": sc,
        }
        for c in range(NCORES)
    ]


def decode_outputs(results):
    fields, cnts = [], []
    for r in results:
        o = np.asarray(r["out"])  # [128, 8]
        fields.append(o[:, 0:NT].T.reshape(NQ))
        cnts.append(o[:, NT : 2 * NT].T.reshape(NQ))
    field = np.concatenate(fields).astype(np.float32)
    cnt = np.concatenate(cnts)
    valid = cnt > 0.5
    return field, valid


def kernel(x_world, voxel_point, voxel_normal, score):
    nc = build_nc()
    in_maps = make_in_maps(x_world, voxel_point, voxel_normal, score)
    res = run_bass_kernel_spmd(nc, in_maps, core_ids=list(range(NCORES)))
    return decode_outputs(res.results)


# revision 18
# speedup vs baseline: 1.2999x; 1.2999x over previous
"""Trainium2 Bass kernel for retrieval_knn (nn_Direct_25701084299719).

For each of N=4096 query points vs M=16384 voxels:
  - top-8 nearest voxels (L2), mean of their normals
  - cosine(mean_normal, voxel_normal) > 0.75 mask
  - score_num = sum(mask); score_sum = sum(score * mask / exp(distance))
  - out = (score_sum / max(score_num, 1)) where score_num != 0 else 0, plus valid

Sharding: data-parallel over queries across 8 NeuronCores (512 queries/core);
voxel tensors replicated. No collectives.

Device algorithm (per core; selection in fp32, streaming math in bf16):
  y[q,m]    = 2<x_q, v_m> - |v_m|^2          (K=4 f32r matmul; desc == nearest)
  top8/idx  = nc.vector.max / max_index      (native per-partition top-8)
  xn        = sum of 8 gathered normals      (indirect-DMA gather, strided adds)
  numth[q,m]= <xn_q, n_m> - 0.75|xn_q||n_m|  (second K=4 f32r matmul)
  mask      = numth > 0  (ACT Sign, count fused via accum_out)
  contrib   = sign * exp(ln s - d), fused multiply+reduce on DVE

Engine split per 1024-col PSUM chunk:
  PE: 2x 512-col f32r matmul   Pool: PSUM->y copy (f32, for selection)
  ACT: Sqrt direct from PSUM   DVE: z = ln s - d (bf16 2x)
then one full-row Exp (accum -> sum es), and in phase C per chunk:
  ACT: Sign from PSUM (accum -> count)   DVE: scalar_tensor_tensor
  prod+partial-reduce (bf16, accum -> sum sg*es)
"""

import sys

for p in ("/opt/trn_rl_repo", "/root/.axon_site/_ro/trn_rl_repo"):
    if p not in sys.path:
        sys.path.insert(0, p)

import numpy as np
from contextlib import ExitStack

import concourse.bass as bass
import concourse.mybir as mybir
from concourse import tile, masks, bacc
from concourse.bass import IndirectOffsetOnAxis
from concourse.bass_utils import run_bass_kernel_spmd

F32 = mybir.dt.float32
BF16 = mybir.dt.bfloat16
F32R = mybir.dt.float32r
U32 = mybir.dt.uint32
AF = mybir.ActivationFunctionType
OP = mybir.AluOpType
AX = mybir.AxisListType

N, M, K = 4096, 16384, 8
NCORES = 8
NQ = N // NCORES          # 512 queries per core
NT = NQ // 128            # 4 query tiles per core
CH = 1024                 # m-chunk (2 PSUM banks)
NCH = M // CH             # 16 chunks
LCH = 4096                # A-table DMA load chunk
NLCH = M // LCH           # 4 loads per table per tile

_nc_cache = {}


def build_nc():
    if "nc" in _nc_cache:
        return _nc_cache["nc"]
    nc = bacc.Bacc("TRN2", target_bir_lowering=False, debug=False)
    x_d = nc.declare_dram_parameter("x", [NQ, 3], F32, isOutput=False)
    vp_d = nc.declare_dram_parameter("voxel_point", [M, 3], F32, isOutput=False)
    vn_d = nc.declare_dram_parameter("voxel_normal", [M, 3], F32, isOutput=False)
    sc_d = nc.declare_dram_parameter("score", [M], F32, isOutput=False)
    out_d = nc.declare_dram_parameter("out", [128, 2 * NT], F32, isOutput=True)
    # Internal DRAM staging. All m-indexed tensors share one fixed voxel
    # permutation m' = j*128+p <-> voxel p*128+j (from the PE-transpose build);
    # reductions over m are permutation-invariant so results are unchanged.
    # 3-way bf16-split dist table (x=a0+a1+a2, v=b0+b1+b2, ss=c0+c1+c2):
    # y = sum over pairs (0,0),(1,0),(0,1),(2,0),(0,2),(1,1) of 2a_i.b_j - ss.
    # Error ~3e-6 (needed: min dist^2 1.3e-5, min top-8 gap ~1e-7), and bf16
    # runs at 1 cycle/row vs 4 for f32 regardless of K.
    KD = 21
    Adist_b = nc.dram_tensor("A_dist_b", [KD, M], BF16)
    Acos = nc.dram_tensor("A_cos", [4, M], F32)
    vn4_dram = nc.dram_tensor("vn4", [M, 4], F32)     # padded normals for gather
    sc16_dram = nc.dram_tensor("sc16", [M], BF16)     # permuted bf16 ln(score)

    with tile.TileContext(nc) as tc, ExitStack() as ctx:
        cpool = ctx.enter_context(tc.tile_pool(name="const", bufs=1))
        big = ctx.enter_context(tc.tile_pool(name="big", bufs=1))
        prep = ctx.enter_context(tc.tile_pool(name="prep", bufs=2))
        chk = ctx.enter_context(tc.tile_pool(name="chk", bufs=2))
        sgp = ctx.enter_context(tc.tile_pool(name="sgp", bufs=3))
        ppb = ctx.enter_context(tc.tile_pool(name="ppb", bufs=3, space="PSUM"))
        ppt = ctx.enter_context(tc.tile_pool(name="ppt", bufs=2, space="PSUM"))
        sm = ctx.enter_context(tc.tile_pool(name="sm", bufs=2))

        ident = cpool.tile([128, 128], F32)
        masks.make_identity(nc, ident[:])

        # ---------------- prep: voxel-side tensors ----------------
        vn_sb = prep.tile([128, 384], F32)
        nc.sync.dma_start(vn_sb[:], vn_d[:].rearrange("(p j) d -> p (j d)", p=128))
        vp_sb = prep.tile([128, 384], F32)
        nc.sync.dma_start(vp_sb[:], vp_d[:].rearrange("(p j) d -> p (j d)", p=128))

        def build_A(src_sb, A_dram, row3_from_sq, keep4_sb=None):
            """A rows 0-2: coordinate d in m' order; row 3 from sum of squares.
            keep4_sb: optionally collect rows 0-2 into a [128, 128*4] row-padded
            layout (gather table: partition j holds voxels j*128..j*128+127)."""
            v3 = src_sb[:].rearrange("p (j d) -> p j d", d=3)
            for d in range(3):
                ps = ppt.tile([128, 128], F32, tag="pt")
                nc.tensor.transpose(ps[:], v3[:, :, d], ident[:])
                tmp = prep.tile([128, 128], F32, tag="atmp")
                nc.scalar.activation(tmp[:], ps[:], AF.Copy)
                nc.sync.dma_start(
                    A_dram[d : d + 1, :].rearrange("o (j p) -> (o j) p", j=128), tmp[:]
                )
                if keep4_sb is not None:
                    k4 = keep4_sb[:].rearrange("p (c e) -> p c e", e=4)
                    nc.vector.tensor_copy(k4[:, :, d], tmp[:])
            sq = prep.tile([128, 384], F32, tag="asq")
            nc.scalar.activation(sq[:], src_sb[:], AF.Square)
            s3 = sq[:].rearrange("p (j d) -> p j d", d=3)
            ss = prep.tile([128, 128], F32, tag="ass")
            nc.vector.tensor_tensor(ss[:], s3[:, :, 0], s3[:, :, 1], OP.add)
            nc.vector.tensor_tensor(ss[:], ss[:], s3[:, :, 2], OP.add)
            r3 = prep.tile([128, 128], F32, tag="ar3")
            row3_from_sq(r3, ss)
            ps = ppt.tile([128, 128], F32, tag="pt")
            nc.tensor.transpose(ps[:], r3[:], ident[:])
            tmp = prep.tile([128, 128], F32, tag="atmp")
            nc.scalar.activation(tmp[:], ps[:], AF.Copy)
            nc.sync.dma_start(
                A_dram[3:4, :].rearrange("o (j p) -> (o j) p", j=128), tmp[:]
            )

        def split3_rows(tmp, rows0, rows1, rows2):
            """3-way bf16 split of a [128,128] f32 m'-layout row; each part is
            exactly bf16. Write part i to DRAM rows in rows_i."""
            rem = tmp
            for rows in (rows0, rows1, rows2):
                p16 = prep.tile([128, 128], BF16, tag="p16")
                nc.vector.tensor_copy(p16[:], rem[:])
                for r in rows:
                    nc.sync.dma_start(
                        Adist_b[r : r + 1, :].rearrange("o (j p) -> (o j) p", j=128),
                        p16[:],
                    )
                if rows is not rows2:
                    pf = prep.tile([128, 128], F32, tag="pf")
                    nc.vector.tensor_copy(pf[:], p16[:])
                    nrem = prep.tile([128, 128], F32, tag="nrem")
                    nc.vector.tensor_tensor(nrem[:], rem[:], pf[:], OP.subtract)
                    rem = nrem

        # dist table row layout (paired lhs part in brackets):
        #   d+0:  b0 [2a0]   d+3:  b0 [2a1]   d+6:  b1 [2a0]
        #   d+9:  b0 [2a2]   d+12: b2 [2a0]   d+15: b1 [2a1]
        #   18/19/20: -|v|^2 c0/c1/c2 [1]
        vp3 = vp_sb[:].rearrange("p (j d) -> p j d", d=3)
        for d in range(3):
            psd = ppt.tile([128, 128], F32, tag="pt")
            nc.tensor.transpose(psd[:], vp3[:, :, d], ident[:])
            tmpd = prep.tile([128, 128], F32, tag="atmp")
            nc.scalar.activation(tmpd[:], psd[:], AF.Copy)
            split3_rows(tmpd, [d, d + 3, d + 9], [d + 6, d + 15], [d + 12])
        sqv = prep.tile([128, 384], F32, tag="asq")
        nc.scalar.activation(sqv[:], vp_sb[:], AF.Square)
        sv3 = sqv[:].rearrange("p (j d) -> p j d", d=3)
        ssv = prep.tile([128, 128], F32, tag="ass")
        nc.vector.tensor_tensor(ssv[:], sv3[:, :, 0], sv3[:, :, 1], OP.add)
        nc.vector.tensor_tensor(ssv[:], ssv[:], sv3[:, :, 2], OP.add)
        nc.vector.tensor_scalar_mul(ssv[:], ssv[:], -1.0)
        psv = ppt.tile([128, 128], F32, tag="pt")
        nc.tensor.transpose(psv[:], ssv[:], ident[:])
        tmpv = prep.tile([128, 128], F32, tag="atmp")
        nc.scalar.activation(tmpv[:], psv[:], AF.Copy)
        split3_rows(tmpv, [18], [19], [20])

        # A_cos row3 = +0.75*|n| = sqrt(0.5625*|n|^2)
        vn4_sb = prep.tile([128, 512], F32)
        nc.vector.memset(vn4_sb[:], 0.0)
        build_A(vn_sb, Acos,
                lambda r3, ss: nc.scalar.activation(r3[:], ss[:], AF.Sqrt, scale=0.5625),
                keep4_sb=vn4_sb)
        # vn4[j*128+p, 0:3] = vn in m' order, rows padded to 16B for the gather
        nc.sync.dma_start(
            vn4_dram[:].rearrange("(j p) e -> j (p e)", j=128), vn4_sb[:]
        )

        # bf16 permuted ln(score) row in DRAM, then partition-broadcast.
        # Folding score into the exponent (exp(ln s - d)) removes one full
        # elementwise multiply from the stream.
        sc_pm = prep.tile([128, 128], F32)
        nc.sync.dma_start(sc_pm[:], sc_d[:].rearrange("(p j) -> p j", p=128))
        ln_pm = prep.tile([128, 128], F32)
        nc.scalar.activation(ln_pm[:], sc_pm[:], AF.Ln)
        psT = ppt.tile([128, 128], F32, tag="pt")
        nc.tensor.transpose(psT[:], ln_pm[:], ident[:])
        scT16 = prep.tile([128, 128], BF16)
        nc.scalar.activation(scT16[:], psT[:], AF.Copy)
        nc.sync.dma_start(sc16_dram[:].rearrange("(j p) -> j p", j=128), scT16[:])
        lns_bc = big.tile([128, M], BF16, tag="sbc")
        nc.sync.dma_start(
            lns_bc[:], sc16_dram[:].rearrange("(o m) -> o m", o=1).partition_broadcast(128)
        )

        # ---------------- prep: query-side tensors ----------------
        xxs, lts = [], []
        for t in range(NT):
            xt = cpool.tile([128, 3], F32, tag=f"xt{t}")
            nc.sync.dma_start(xt[:], x_d[t * 128 : (t + 1) * 128, :])
            sqx = sm.tile([128, 3], F32, tag="sqx")
            nc.scalar.activation(sqx[:], xt[:], AF.Square)
            xx = cpool.tile([128, 1], F32, tag=f"xx{t}")
            nc.vector.tensor_reduce(xx[:], sqx[:], AX.X, OP.add)
            # lhsT cols pre-transpose: [a0,a1,a0,a2,a0,a1,(0.5)x3]; 3-way bf16
            # split of x (each part exactly bf16), Copy(scale=2) makes 2a_i/1
            a0 = sm.tile([128, 3], BF16, tag="a0")
            nc.vector.tensor_copy(a0[:], xt[:])
            a0f = sm.tile([128, 3], F32, tag="a0f")
            nc.vector.tensor_copy(a0f[:], a0[:])
            r1 = sm.tile([128, 3], F32, tag="r1")
            nc.vector.tensor_tensor(r1[:], xt[:], a0f[:], OP.subtract)
            a1 = sm.tile([128, 3], BF16, tag="a1")
            nc.vector.tensor_copy(a1[:], r1[:])
            a1f = sm.tile([128, 3], F32, tag="a1f")
            nc.vector.tensor_copy(a1f[:], a1[:])
            a2f = sm.tile([128, 3], F32, tag="a2f")
            nc.vector.tensor_tensor(a2f[:], r1[:], a1f[:], OP.subtract)
            xt21 = sm.tile([128, KD], F32, tag="xt21")
            nc.vector.tensor_copy(xt21[:, 0:3], a0f[:])
            nc.vector.tensor_copy(xt21[:, 3:6], a1f[:])
            nc.vector.tensor_copy(xt21[:, 6:9], a0f[:])
            nc.vector.tensor_copy(xt21[:, 9:12], a2f[:])
            nc.vector.tensor_copy(xt21[:, 12:15], a0f[:])
            nc.vector.tensor_copy(xt21[:, 15:18], a1f[:])
            nc.vector.memset(xt21[:, 18:21], 0.5)
            ps = ppt.tile([128, 128], F32, tag="pt")
            nc.tensor.transpose(ps[0:KD, 0:128], xt21[:], ident[:])
            ltf = sm.tile([KD, 128], F32, tag="ltf")
            nc.scalar.activation(ltf[:], ps[0:KD, 0:128], AF.Copy, scale=2.0)
            lt = cpool.tile([KD, 128], BF16, tag=f"lt{t}")
            nc.vector.tensor_copy(lt[:], ltf[:])
            xxs.append(xx)
            lts.append(lt)

        out_sb = cpool.tile([128, 2 * NT], F32)

        # ---------------- main loop over query tiles ----------------
        for t in range(NT):
            # Phase A: y = 2<x,v> - |v|^2 (f32r matmul), streamed per chunk:
            #   Pool copies PSUM->y (selection input), ACT computes
            #   d = sqrt(xx - y) straight from PSUM, DVE z = ln s - d.
            y = big.tile([128, M], F32, tag="y")
            dfull = big.tile([128, M], BF16, tag="dfull")
            # per-quarter top-8 candidates, computed inside the A-stream so
            # only the tiny merge + one find_index8 remain on the critical path
            v32 = sm.tile([128, 32], F32, tag="v32")
            QS = M // 4
            for lc in range(NLCH):
                ra = chk.tile([KD, LCH], BF16, tag="ra")
                nc.sync.dma_start(
                    ra[:], Adist_b[:, lc * LCH : (lc + 1) * LCH]
                )
                for cc in range(LCH // CH):
                    c = lc * (LCH // CH) + cc
                    cs = slice(c * CH, (c + 1) * CH)
                    ps = ppb.tile([128, CH], F32, tag="pm")
                    for h in range(2):
                        nc.tensor.matmul(
                            ps[:, h * 512 : (h + 1) * 512],
                            lhsT=lts[t][:],
                            rhs=ra[:, cc * CH + h * 512 : cc * CH + (h + 1) * 512],
                            start=True, stop=True,
                        )
                    # Pool/GpSimd and DMA can't read PSUM; alternate the
                    # selection copy between ACT and DVE to balance load.
                    # Sqrt+bias must read SBUF: the PSUM-input activation
                    # drops the bias on HW (sim diverges), giving sqrt(-y).
                    if c % 2 == 0:
                        nc.scalar.activation(y[:, cs], ps[:], AF.Copy)
                    else:
                        nc.vector.tensor_copy(y[:, cs], ps[:])
                    nc.scalar.activation(
                        dfull[:, cs], y[:, cs], AF.Sqrt, bias=xxs[t][:], scale=-1.0
                    )
                    nc.vector.tensor_tensor(
                        dfull[:, cs], lns_bc[:, cs], dfull[:, cs], OP.subtract
                    )
                    if c % 4 == 3:
                        q = c // 4
                        nc.vector.max(v32[:, q * 8 : (q + 1) * 8],
                                      y[:, q * QS : (q + 1) * QS])

            # One full-row exp: es = s * exp(-d); accE = sum(es) for sign trick
            accE = sm.tile([128, 1], F32, tag="accE")
            nc.scalar.activation(dfull[:], dfull[:], AF.Exp, accum_out=accE[:])

            # Selection merge: global top8 = top8 of the 32 quarter candidates
            # (any global top-8 value is in its quarter's top-8), then one
            # full-row index search.
            top8 = sm.tile([128, 8], F32, tag="top8")
            nc.vector.max(top8[:], v32[:])
            idx8 = sm.tile([128, 8], U32, tag="idx8")
            nc.vector.max_index(idx8[:], top8[:], y[:])
            # HW DGE consumes one offset per partition per instruction, so
            # issue one gather per neighbor rank.
            g = sm.tile([128, 32], F32, tag="gat")
            g3 = g[:].rearrange("p (i e) -> p i e", e=4)
            for i in range(8):
                nc.gpsimd.indirect_dma_start(
                    g3[:, i, :], None, vn4_dram[:],
                    IndirectOffsetOnAxis(ap=idx8[:, i : i + 1], axis=0),
                )
            # xn = sum of the 8 gathered normal rows (tree of strided adds)
            h16 = sm.tile([128, 16], F32, tag="h16")
            nc.vector.tensor_tensor(h16[:], g[:, 0:16], g[:, 16:32], OP.add)
            h8 = sm.tile([128, 8], F32, tag="h8")
            nc.vector.tensor_tensor(h8[:], h16[:, 0:8], h16[:, 8:16], OP.add)
            xn4 = sm.tile([128, 4], F32, tag="xn4")
            nc.vector.tensor_tensor(xn4[:], h8[:, 0:4], h8[:, 4:8], OP.add)

            # lhsT for cos matmul: [xn0;xn1;xn2;-|xn|] (xn = 8*mean, scale-free)
            sqn = sm.tile([128, 3], F32, tag="sqn")
            nc.vector.tensor_tensor(sqn[:], xn4[:, 0:3], xn4[:, 0:3], OP.mult)
            nrm2 = sm.tile([128, 1], F32, tag="nrm2")
            nc.vector.tensor_reduce(nrm2[:], sqn[:], AX.X, OP.add)
            nc.scalar.activation(xn4[:, 3:4], nrm2[:], AF.Sqrt)
            nc.vector.tensor_scalar_mul(xn4[:, 3:4], xn4[:, 3:4], -1.0)
            psl = ppt.tile([128, 128], F32, tag="pt")
            nc.tensor.transpose(psl[0:4, 0:128], xn4[:], ident[:])
            ltc = sm.tile([4, 128], F32R, tag="ltc")
            nc.vector.tensor_copy(ltc[:], psl[0:4, 0:128])

            # Phase C per chunk: cos matmul -> ACT Sign (accum: count) ->
            # DVE fused prod+partial-reduce (accum: sum sg*es).
            # count = (sum(sg) + M)/2 ; ssum = (sum(sg*es) + sum(es))/2
            accSg = sm.tile([128, NCH], F32, tag="accSg")
            accS = sm.tile([128, NCH], F32, tag="accS")
            for lc in range(NLCH):
                rc = chk.tile([4, LCH], F32R, tag="ra")
                nc.sync.dma_start(
                    rc[:], Acos[:, lc * LCH : (lc + 1) * LCH].bitcast(F32R)
                )
                for cc in range(LCH // CH):
                    c = lc * (LCH // CH) + cc
                    cs = slice(c * CH, (c + 1) * CH)
                    psn = ppb.tile([128, CH], F32, tag="pm")
                    for h in range(2):
                        nc.tensor.matmul(
                            psn[:, h * 512 : (h + 1) * 512],
                            lhsT=ltc[:],
                            rhs=rc[:, cc * CH + h * 512 : cc * CH + (h + 1) * 512],
                            start=True, stop=True,
                        )
                    sg = sgp.tile([128, CH], BF16, tag="sg")
                    nc.scalar.activation(
                        sg[:], psn[:], AF.Sign, accum_out=accSg[:, c : c + 1]
                    )
                    # fused prod+partial-reduce (TensorScalarPtr is not legal
                    # on Pool, so it stays on DVE)
                    prod = sgp.tile([128, CH], BF16, tag="prod")
                    nc.vector.scalar_tensor_tensor(
                        prod[:], sg[:], 1.0, dfull[:, cs], OP.mult, OP.mult,
                        accum_out=accS[:, c : c + 1],
                    )

            cnt = sm.tile([128, 1], F32, tag="cnt")
            nc.vector.tensor_reduce(cnt[:], accSg[:], AX.X, OP.add)
            nc.vector.tensor_scalar(cnt[:], cnt[:], float(M), 0.5, OP.add, OP.mult)
            ssum = sm.tile([128, 1], F32, tag="ssum")
            nc.vector.tensor_reduce(ssum[:], accS[:], AX.X, OP.add)
            nc.vector.tensor_tensor(ssum[:], ssum[:], accE[:], OP.add)
            nc.vector.tensor_scalar_mul(ssum[:], ssum[:], 0.5)
            den = sm.tile([128, 1], F32, tag="den")
            nc.vector.tensor_scalar_max(den[:], cnt[:], 1.0)
            rden = sm.tile([128, 1], F32, tag="rden")
            nc.vector.reciprocal(rden[:], den[:])
            vld = sm.tile([128, 1], F32, tag="vld")
            nc.vector.tensor_scalar(vld[:], cnt[:], 0.5, None, OP.is_gt)
            fld = sm.tile([128, 1], F32, tag="fld")
            nc.vector.tensor_tensor(fld[:], ssum[:], rden[:], OP.mult)
            nc.vector.tensor_tensor(fld[:], fld[:], vld[:], OP.mult)
            nc.vector.tensor_copy(out_sb[:, t : t + 1], fld[:])
            nc.vector.tensor_copy(out_sb[:, NT + t : NT + t + 1], cnt[:])

        nc.sync.dma_start(out_d[:, :], out_sb[:])

    nc.compile()
    _nc_cache["nc"] = nc
    return nc


def make_in_maps(x_world, voxel_point, voxel_normal, score):
    x = np.ascontiguousarray(np.asarray(x_world, np.float32).reshape(N, 3))
    vp = np.ascontiguousarray(np.asarray(voxel_point, np.float32).reshape(M, 3))
    vn = np.ascontiguousarray(np.asarray(voxel_normal, np.float32).reshape(M, 3))
    sc = np.ascontiguousarray(np.asarray(score, np.float32).reshape(M))
    return [
        {
            "x": np.ascontiguousarray(x[c * NQ : (c + 1) * NQ]),
            "voxel_point": vp,
            "voxel_normal": vn,
            "score": sc,
        }
        for c in range(NCORES)
    ]


def decode_outputs(results):
    fields, cnts = [], []
    for r in results:
        o = np.asarray(r["out"])  # [128, 8]
        fields.append(o[:, 0:NT].T.reshape(NQ))
        cnts.append(o[:, NT : 2 * NT].T.reshape(NQ))
    field = np.concatenate(fields).astype(np.float32)
    cnt = np.concatenate(cnts)
    valid = cnt > 0.5
    return field, valid


def kernel(x_world, voxel_point, voxel_normal, score):
    nc = build_nc()
    in_maps = make_in_maps(x_world, voxel_point, voxel_normal, score)
    res = run_bass_kernel_spmd(nc, in_maps, core_ids=list(range(NCORES)))
    return decode_outputs(res.results)
